# revision 11
# baseline (speedup 1.0000x reference)
"""Trainium2 Bass kernel for nn_AttentionBlock (B=16, T=2048, C=512, KS=VS=16).

Reference semantics (note the unusual softmax axis):
    q = X @ Wq.T + bq ; k = X @ Wk.T + bk ; v = X @ Wv.T + bv      [B,T,16]
    scores[b,j,i] = q[b,j] . k[b,i], masked -inf where i > j
    attn = softmax(scores / 4, axis=1)        # normalized over j (query axis)!
    out[b,j] = sum_i attn[b,j,i] v[b,i]
    return concat([X, out], -1)               # [B,T,528]

Kernel strategy (pure batch data-parallel, 2 batches per core, 8 cores):
  * Work in transposed score layout ST[i, j] (key i on partitions): the
    softmax norm over j becomes a free-axis reduction, fused into the exp
    on the scalar engine via accum_out.  The 1/denom factor depends only on
    the PV contraction index i, so it is folded into v.
  * q-bias cancels inside softmax-over-j (constant in j for fixed i) and is
    dropped; k-bias and v-bias are applied.
  * Causal mask applied by accumulating I128 @ maskneg(-1e9) into the score
    psum on the PE; exp underflows to exactly 0.
  * PE matmuls run as float32r (full-rate fp32 streaming).
  * Output written as [X | out] assembled in one SBUF tile per batch, single
    contiguous DMA per batch each way.
"""

import sys

if "/opt/trn_rl_repo" not in sys.path:
    sys.path.insert(0, "/opt/trn_rl_repo")

import numpy as np

from contextlib import ExitStack

import concourse.bacc as bacc
import concourse.tile as tile
from concourse import mybir
from concourse.bass_utils import run_bass_kernel_spmd

B, T, C = 16, 2048, 512
KS = 16
NCORES = 8
NB = B // NCORES            # batches per core
NT = T // 128               # 16 t-tiles per batch
OUTW = C + KS               # 528
F32 = mybir.dt.float32
F32R = mybir.dt.float32r
NEG = -1.0e9

_cache = {}


def _segments(w, seg=1024):
    off, out = 0, []
    while off < w:
        out.append((off, min(seg, w - off)))
        off += min(seg, w - off)
    return out


def _chunks(w, ch=512):
    off, out = 0, []
    while off < w:
        out.append((off, min(ch, w - off)))
        off += min(ch, w - off)
    return out


def _build_program():
    nc = bacc.Bacc("TRN2", target_bir_lowering=False, debug=False)

    x_t = nc.dram_tensor("x", [NB, T, C], F32R, kind="ExternalInput")
    w1_t = nc.dram_tensor("w1", [C, 128], F32R, kind="ExternalInput")
    w2_t = nc.dram_tensor("w2", [C, KS], F32R, kind="ExternalInput")
    b48_t = nc.dram_tensor("b48", [48, 1], F32, kind="ExternalInput")
    bk_t = nc.dram_tensor("bk16", [KS, 1], F32, kind="ExternalInput")
    mneg_t = nc.dram_tensor("mneg", [128, 512], F32R, kind="ExternalInput")
    idn_t = nc.dram_tensor("idn", [128, 128], F32, kind="ExternalInput")
    idnr_t = nc.dram_tensor("idnr", [128, 128], F32R, kind="ExternalInput")
    y_t = nc.dram_tensor("y", [NB, T, OUTW], F32R, kind="ExternalOutput")

    with tile.TileContext(nc) as tc, ExitStack() as ctx:
        consts = ctx.enter_context(tc.tile_pool(name="consts", bufs=1))
        outp = ctx.enter_context(tc.tile_pool(name="outp", bufs=2))
        xtp = ctx.enter_context(tc.tile_pool(name="xtp", bufs=1))
        qkp = ctx.enter_context(tc.tile_pool(name="qkp", bufs=2))
        vp_pool = ctx.enter_context(tc.tile_pool(name="vpp", bufs=2))
        pp = ctx.enter_context(tc.tile_pool(name="pp", bufs=2))
        smalls = ctx.enter_context(tc.tile_pool(name="smalls", bufs=4))
        osb = ctx.enter_context(tc.tile_pool(name="osb", bufs=2))
        ps = ctx.enter_context(tc.tile_pool(name="ps", bufs=2, space="PSUM"))
        ps_ot = ctx.enter_context(tc.tile_pool(name="ps_ot", bufs=1, space="PSUM"))

        # ---- constants ----
        w1_sb = consts.tile([128, 4, 128], F32R)     # [c_in_chunk, c_chunk, m]
        nc.sync.dma_start(out=w1_sb, in_=w1_t[:].rearrange("(cc p) m -> p cc m", p=128))
        w2_sb = consts.tile([128, 4, KS], F32R)
        nc.sync.dma_start(out=w2_sb, in_=w2_t[:].rearrange("(cc p) m -> p cc m", p=128))
        b48_sb = consts.tile([48, 1], F32)
        nc.sync.dma_start(out=b48_sb, in_=b48_t[:])
        bk_sb = consts.tile([KS, 1], F32)
        nc.sync.dma_start(out=bk_sb, in_=bk_t[:])
        mneg_sb = consts.tile([128, 512], F32R)
        nc.sync.dma_start(out=mneg_sb, in_=mneg_t[:])
        idn_sb = consts.tile([128, 128], F32)
        nc.sync.dma_start(out=idn_sb, in_=idn_t[:])
        idnr_sb = consts.tile([128, 128], F32R)
        nc.sync.dma_start(out=idnr_sb, in_=idnr_t[:])

        for b in range(NB):
            # OUT holds the assembled [X | attn_out] rows: 16 tiles of [128, 528].
            OUT = outp.tile([128, NT * OUTW], F32R, tag="OUT")
            OUTv = OUT.rearrange("p (n f) -> p n f", f=OUTW)
            nc.sync.dma_start(
                out=OUTv[:, :, 0:C],
                in_=x_t[b].rearrange("(n p) c -> p n c", p=128),
            )

            # ---- transpose X into XT chunks; project q/v and k ----
            xts = [xtp.tile([128, T], F32R, tag=f"xt{c}", name=f"xt{c}") for c in range(4)]
            sb1 = qkp.tile([48, T], F32R, tag="sb1")   # rows 0:16 qT, 32:48 vT
            sb2 = qkp.tile([KS, T], F32R, tag="sb2")   # kT
            for tq in range(4):
                for c in range(4):
                    ptr = ps.tile([128, 512], F32R, tag="st", name="ptr")
                    for k in range(4):
                        tb = 4 * tq + k
                        nc.tensor.matmul(
                            ptr[:, 128 * k:128 * (k + 1)],
                            lhsT=OUTv[:, tb, 128 * c:128 * (c + 1)],
                            rhs=idnr_sb,
                            is_transpose=True,
                            start=(k == 0),
                            stop=(k == 3),
                        )
                    nc.vector.tensor_copy(out=xts[c][:, 512 * tq:512 * (tq + 1)], in_=ptr)
                # projections for this t-chunk
                p1 = ps.tile([48, 512], F32, tag="st", name="p1")
                for c in range(4):
                    nc.tensor.matmul(
                        p1,
                        lhsT=w1_sb[:, c, 0:48],
                        rhs=xts[c][:, 512 * tq:512 * (tq + 1)],
                        start=(c == 0),
                        stop=(c == 3),
                    )
                nc.vector.tensor_scalar_add(
                    out=sb1[:, 512 * tq:512 * (tq + 1)], in0=p1, scalar1=b48_sb
                )
                p2 = ps.tile([KS, 512], F32, tag="st", name="p2")
                for c in range(4):
                    nc.tensor.matmul(
                        p2,
                        lhsT=w2_sb[:, c, :],
                        rhs=xts[c][:, 512 * tq:512 * (tq + 1)],
                        start=(c == 0),
                        stop=(c == 3),
                    )
                nc.vector.tensor_scalar_add(
                    out=sb2[:, 512 * tq:512 * (tq + 1)], in0=p2, scalar1=bk_sb
                )

            # ---- v natural layout [i, 16] via PE transposes of vT = sb1[32:48] ----
            v_all = vp_pool.tile([128, NT * KS], F32, tag="v_all")
            for n in range(NT):
                pv = ps.tile([128, KS], F32R, tag="st", name="pv")
                nc.tensor.matmul(
                    pv,
                    lhsT=sb1[32:48, 128 * n:128 * (n + 1)],
                    rhs=idnr_sb[32:48, 32:48],
                    is_transpose=True,
                )
                nc.vector.tensor_copy(out=v_all[:, KS * n:KS * (n + 1)], in_=pv)

            # ---- fused ST -> exp+rowsum -> PV loop over key tiles ----
            outT = ps_ot.tile([KS, T], F32, tag="outT")

            def emit_pv(it, vprime, P):
                j0 = 128 * it
                for jc in range(4):
                    if it > 4 * jc + 3:
                        continue
                    if it <= 4 * jc:
                        ooff, ncols, poff = 0, 512, 512 * jc - j0
                    else:
                        ooff = j0 - 512 * jc
                        ncols = 512 - ooff
                        poff = 0
                    nc.tensor.matmul(
                        outT[:, 512 * jc + ooff: 512 * jc + ooff + ncols],
                        lhsT=vprime,
                        rhs=P[:, poff:poff + ncols],
                        start=(it == 0),
                        stop=(it == 4 * jc + 3),
                    )

            pending = None
            for it in range(NT):
                W = T - 128 * it          # panel width (cols j in [128*it, T))
                j0 = 128 * it
                P = pp.tile([128, T], F32R, tag="P")
                segs = _segments(W)
                acc = smalls.tile([128, 2], F32, tag="acc")
                for si, (soff, sw) in enumerate(segs):
                    stps = ps.tile([128, 1024], F32, tag="st", name="stps")
                    for (coff, cw) in _chunks(sw):
                        first = (soff + coff == 0)
                        nc.tensor.matmul(
                            stps[:, coff:coff + cw],
                            lhsT=sb2[:, j0:j0 + 128],
                            rhs=sb1[0:16, j0 + soff + coff: j0 + soff + coff + cw],
                            start=True,
                            stop=not first,
                        )
                        if first:
                            nc.tensor.matmul(
                                stps[:, 0:cw],
                                lhsT=idnr_sb,
                                rhs=mneg_sb[:, 0:cw],
                                start=False,
                                stop=True,
                            )
                    nc.scalar.activation(
                        out=P[:, soff:soff + sw],
                        in_=stps[:, 0:sw],
                        func=mybir.ActivationFunctionType.Exp,
                        scale=0.25,
                        accum_out=acc[:, si:si + 1],
                    )
                if pending is not None:
                    emit_pv(*pending)
                den = smalls.tile([128, 1], F32, tag="den")
                if len(segs) > 1:
                    nc.vector.reduce_sum(den, acc[:, 0:len(segs)], axis=mybir.AxisListType.X)
                else:
                    nc.vector.tensor_copy(out=den, in_=acc[:, 0:1])
                rd = smalls.tile([128, 1], F32, tag="rd")
                nc.vector.reciprocal(rd, den)
                vprime = smalls.tile([128, KS], F32R, tag="vprime")
                nc.vector.tensor_scalar_mul(
                    out=vprime, in0=v_all[:, KS * it:KS * (it + 1)], scalar1=rd
                )
                pending = (it, vprime, P)
            emit_pv(*pending)

            # ---- epilogue: transpose outT back to [t, 16] and finish rows ----
            oT = osb.tile([KS, T], F32R, tag="oT")
            nc.vector.tensor_copy(out=oT, in_=outT)
            for n in range(NT):
                po = ps.tile([128, KS], F32R, tag="st", name="po")
                nc.tensor.matmul(
                    po,
                    lhsT=oT[:, 128 * n:128 * (n + 1)],
                    rhs=idnr_sb[0:KS, 0:KS],
                    is_transpose=True,
                )
                nc.vector.tensor_copy(out=OUTv[:, n, C:OUTW], in_=po)
            nc.sync.dma_start(
                out=y_t[b].rearrange("(n p) f -> p n f", p=128),
                in_=OUTv,
            )

    nc.compile()
    return nc


def _host_params(Wq, bq, Wk, bk, Wv, bv):
    w1 = np.zeros((C, 128), dtype=np.float32)
    w1[:, 0:KS] = Wq.T
    w1[:, 32:48] = Wv.T
    w2 = np.ascontiguousarray(Wk.T.astype(np.float32))
    b48 = np.zeros((48, 1), dtype=np.float32)
    b48[32:48, 0] = bv
    bk16 = np.ascontiguousarray(bk.astype(np.float32).reshape(KS, 1))
    p = np.arange(128)[:, None]
    f = np.arange(512)[None, :]
    mneg = np.where(f < p, np.float32(NEG), np.float32(0.0)).astype(np.float32)
    idn = np.eye(128, dtype=np.float32)
    return w1, w2, b48, bk16, mneg, idn


def run(inputs, Wq, bq, Wk, bk, Wv, bv, trace=False):
    """Run on 8 cores; returns (out [B,T,OUTW], BassKernelResults)."""
    if "nc" not in _cache:
        _cache["nc"] = _build_program()
    nc = _cache["nc"]
    w1, w2, b48, bk16, mneg, idn = _host_params(
        np.asarray(Wq), np.asarray(bq), np.asarray(Wk),
        np.asarray(bk), np.asarray(Wv), np.asarray(bv))
    x = np.ascontiguousarray(np.asarray(inputs), dtype=np.float32)
    in_maps = []
    for core in range(NCORES):
        in_maps.append({
            "x": x[NB * core: NB * (core + 1)],
            "w1": w1, "w2": w2, "b48": b48, "bk16": bk16,
            "mneg": mneg, "idn": idn, "idnr": idn,
        })
    res = run_bass_kernel_spmd(nc, in_maps, core_ids=list(range(NCORES)), trace=trace)
    out = np.concatenate([res.results[i]["y"] for i in range(NCORES)], axis=0)
    return out, res


def kernel(inputs, Wq, bq, Wk, bk, Wv, bv):
    out, _ = run(inputs, Wq, bq, Wk, bk, Wv, bv, trace=False)
    return out


# revision 12
# speedup vs baseline: 1.2026x; 1.2026x over previous
"""Trainium2 Bass kernel for nn_AttentionBlock (B=16, T=2048, C=512, KS=VS=16).

Reference semantics (note the unusual softmax axis):
    q = X @ Wq.T + bq ; k = X @ Wk.T + bk ; v = X @ Wv.T + bv      [B,T,16]
    scores[b,j,i] = q[b,j] . k[b,i], masked -inf where i > j
    attn = softmax(scores / 4, axis=1)        # normalized over j (query axis)!
    out[b,j] = sum_i attn[b,j,i] v[b,i]
    return concat([X, out], -1)               # [B,T,528]

Kernel strategy (pure batch data-parallel, 2 batches per core, 8 cores):
  * Transposed score layout ST[i, j] (key i on partitions): the softmax
    norm over j is a free-axis reduction, fused into the exp on the scalar
    engine via accum_out.  1/denom depends only on the PV contraction index
    i, so it is folded into v (v' = v/denom) -- no normalization pass.
  * q-bias cancels inside softmax-over-j (constant in j for fixed i) and is
    dropped; k-bias and v-bias are applied.
  * Causal mask applied by one K=128 PE matmul accumulating
    I128 @ maskneg(-3e4) into the first 128 score columns; exp underflows
    to exactly 0.
  * All attention matmuls in fp16 (same 10-bit mantissa as tf32) which
    allows PE tiling: ST runs 2x packed on the array diagonal, PV runs 4x
    packed across column groups.  Scores/outputs accumulate in fp32 PSUM.
  * X passthrough stays fp32 end-to-end (bit-exact); a gpsimd cast makes
    the fp16 copy feeding the projections.
"""

import sys

if "/opt/trn_rl_repo" not in sys.path:
    sys.path.insert(0, "/opt/trn_rl_repo")

import numpy as np

from contextlib import ExitStack

import concourse.bacc as bacc
import concourse.tile as tile
from concourse import mybir
from concourse.bass_utils import run_bass_kernel_spmd

B, T, C = 16, 2048, 512
KS = 16
NCORES = 8
NB = B // NCORES            # batches per core
NT = T // 128               # 16 t-tiles per batch
OUTW = C + KS               # 528
F32 = mybir.dt.float32
F16 = mybir.dt.float16
NEG = -30000.0

_cache = {}


def _segments(w, seg=1024):
    off, out = 0, []
    while off < w:
        out.append((off, min(seg, w - off)))
        off += min(seg, w - off)
    return out


def _chunks(w, ch=512):
    off, out = 0, []
    while off < w:
        out.append((off, min(ch, w - off)))
        off += min(ch, w - off)
    return out


def _build_program():
    nc = bacc.Bacc("TRN2", target_bir_lowering=False, debug=False)

    x_t = nc.dram_tensor("x", [NB, T, C], F32, kind="ExternalInput")
    wq_t = nc.dram_tensor("wq", [C, 128], F16, kind="ExternalInput")
    wk_t = nc.dram_tensor("wk", [C, 128], F16, kind="ExternalInput")
    bq_t = nc.dram_tensor("bq128", [128, 1], F32, kind="ExternalInput")
    bk_t = nc.dram_tensor("bk128", [128, 1], F32, kind="ExternalInput")
    mneg_t = nc.dram_tensor("mneg", [128, 128], F16, kind="ExternalInput")
    idh_t = nc.dram_tensor("idh", [128, 128], F16, kind="ExternalInput")
    y_t = nc.dram_tensor("y", [NB, T, OUTW], F32, kind="ExternalOutput")

    with tile.TileContext(nc) as tc, ExitStack() as ctx:
        consts = ctx.enter_context(tc.tile_pool(name="consts", bufs=1))
        outp = ctx.enter_context(tc.tile_pool(name="outp", bufs=2))
        xhp = ctx.enter_context(tc.tile_pool(name="xhp", bufs=2))
        xtp = ctx.enter_context(tc.tile_pool(name="xtp", bufs=1))
        qkp = ctx.enter_context(tc.tile_pool(name="qkp", bufs=2))
        vp_pool = ctx.enter_context(tc.tile_pool(name="vpp", bufs=2))
        pp = ctx.enter_context(tc.tile_pool(name="pp", bufs=2))
        smalls = ctx.enter_context(tc.tile_pool(name="smalls", bufs=4))
        osb = ctx.enter_context(tc.tile_pool(name="osb", bufs=2))
        ps = ctx.enter_context(tc.tile_pool(name="ps", bufs=3, space="PSUM"))
        ps_ot = ctx.enter_context(tc.tile_pool(name="ps_ot", bufs=1, space="PSUM"))

        # ---- constants ----
        wq_sb = consts.tile([128, 4, 128], F16)   # cols: q@0-15, vT@32-47, q@64-79
        nc.sync.dma_start(out=wq_sb, in_=wq_t[:].rearrange("(cc p) m -> p cc m", p=128))
        wk_sb = consts.tile([128, 4, 128], F16)   # cols: k@0-15, k@64-79
        nc.sync.dma_start(out=wk_sb, in_=wk_t[:].rearrange("(cc p) m -> p cc m", p=128))
        bq_sb = consts.tile([128, 1], F32)
        nc.sync.dma_start(out=bq_sb, in_=bq_t[:])
        bk_sb = consts.tile([128, 1], F32)
        nc.sync.dma_start(out=bk_sb, in_=bk_t[:])
        mneg_sb = consts.tile([128, 128], F16)
        nc.sync.dma_start(out=mneg_sb, in_=mneg_t[:])
        idh_sb = consts.tile([128, 128], F16)
        nc.sync.dma_start(out=idh_sb, in_=idh_t[:])

        for b in range(NB):
            # OUT holds the assembled [X | attn_out] rows: 16 tiles of [128, 528].
            OUT = outp.tile([128, NT * OUTW], F32, tag="OUT")
            OUTv = OUT.rearrange("p (n f) -> p n f", f=OUTW)
            nc.sync.dma_start(
                out=OUTv[:, :, 0:C],
                in_=x_t[b].rearrange("(n p) c -> p n c", p=128),
            )

            # fp16 copy of X for the projection path (gpsimd; otherwise idle)
            Xh = xhp.tile([128, NT * C], F16, tag="Xh")
            Xhv = Xh.rearrange("p (n f) -> p n f", f=C)
            for tq in range(4):
                nc.gpsimd.tensor_copy(
                    out=Xhv[:, 4 * tq:4 * (tq + 1), :],
                    in_=OUTv[:, 4 * tq:4 * (tq + 1), 0:C],
                )

            # ---- transpose Xh into XT chunks; project q/v and k ----
            xts = [xtp.tile([128, T], F16, tag=f"xt{c}", name=f"xt{c}") for c in range(4)]
            sbq = qkp.tile([128, T], F16, tag="sbq")  # q@0-15, vT@32-47, q@64-79
            sbk = qkp.tile([128, T], F16, tag="sbk")  # k@0-15, k@64-79
            for tq in range(4):
                for c in range(4):
                    ptr = ps.tile([128, 512], F16, tag="st", name="ptr")
                    for k in range(4):
                        tb = 4 * tq + k
                        nc.tensor.matmul(
                            ptr[:, 128 * k:128 * (k + 1)],
                            lhsT=Xhv[:, tb, 128 * c:128 * (c + 1)],
                            rhs=idh_sb,
                            is_transpose=True,
                            start=(k == 0),
                            stop=(k == 3),
                        )
                    nc.vector.tensor_copy(out=xts[c][:, 512 * tq:512 * (tq + 1)], in_=ptr)
                pq = ps.tile([128, 512], F32, tag="st", name="pq")
                for c in range(4):
                    nc.tensor.matmul(
                        pq,
                        lhsT=wq_sb[:, c, :],
                        rhs=xts[c][:, 512 * tq:512 * (tq + 1)],
                        start=(c == 0),
                        stop=(c == 3),
                    )
                nc.vector.tensor_scalar_add(
                    out=sbq[:, 512 * tq:512 * (tq + 1)], in0=pq, scalar1=bq_sb
                )
                pk = ps.tile([128, 512], F32, tag="st", name="pk")
                for c in range(4):
                    nc.tensor.matmul(
                        pk,
                        lhsT=wk_sb[:, c, :],
                        rhs=xts[c][:, 512 * tq:512 * (tq + 1)],
                        start=(c == 0),
                        stop=(c == 3),
                    )
                nc.vector.tensor_scalar_add(
                    out=sbk[:, 512 * tq:512 * (tq + 1)], in0=pk, scalar1=bk_sb
                )

            # ---- v natural layout [i, 16] via PE transposes of vT = sbq[32:48] ----
            v_all = vp_pool.tile([128, NT * KS], F16, tag="v_all")
            for n in range(NT):
                pv = ps.tile([128, KS], F16, tag="st", name="pv")
                nc.tensor.matmul(
                    pv,
                    lhsT=sbq[32:48, 128 * n:128 * (n + 1)],
                    rhs=idh_sb[32:48, 32:48],
                    is_transpose=True,
                )
                nc.vector.tensor_copy(out=v_all[:, KS * n:KS * (n + 1)], in_=pv)

            # ---- fused ST -> exp+rowsum -> PV(pipelined) loop over key tiles ----
            outT = ps_ot.tile([128, 512], F32, tag="outT")

            def emit_pv(it, vprime, P):
                j0 = 128 * it
                for jc in range(4):
                    if it > 4 * jc + 3:
                        continue
                    if it <= 4 * jc:
                        ooff, ncols, poff = 0, 512, 512 * jc - j0
                    else:
                        ooff = j0 - 512 * jc
                        ncols = 512 - ooff
                        poff = 0
                    nc.tensor.matmul(
                        outT[32 * jc:32 * jc + KS, ooff:ooff + ncols],
                        lhsT=vprime,
                        rhs=P[:, poff:poff + ncols],
                        start=(it == 0),
                        stop=(it == 4 * jc + 3),
                        tile_position=(0, 32 * jc),
                    )

            pending = None
            for it in range(NT):
                W = T - 128 * it          # panel width (cols j in [128*it, T))
                j0 = 128 * it
                P = pp.tile([128, T], F16, tag="P")
                segs = _segments(W)
                acc = smalls.tile([128, 2], F32, tag="acc")
                for si, (soff, sw) in enumerate(segs):
                    stps = ps.tile([128, 1024], F32, tag="st", name="stps")
                    for (coff, cw) in _chunks(sw):
                        first = (soff + coff == 0)
                        for s in range(2):
                            nc.tensor.matmul(
                                stps[64 * s:64 * (s + 1), coff:coff + cw],
                                lhsT=sbk[64 * s:64 * s + KS,
                                         j0 + 64 * s: j0 + 64 * s + 64],
                                rhs=sbq[64 * s:64 * s + KS,
                                        j0 + soff + coff: j0 + soff + coff + cw],
                                start=True,
                                stop=not first,
                                tile_position=(64 * s, 64 * s),
                            )
                        if first:
                            mw = min(cw, 128)
                            nc.tensor.matmul(
                                stps[:, 0:mw],
                                lhsT=idh_sb,
                                rhs=mneg_sb[:, 0:mw],
                                start=False,
                                stop=True,
                            )
                    nc.scalar.activation(
                        out=P[:, soff:soff + sw],
                        in_=stps[:, 0:sw],
                        func=mybir.ActivationFunctionType.Exp,
                        scale=0.25,
                        accum_out=acc[:, si:si + 1],
                    )
                if pending is not None:
                    emit_pv(*pending)
                den = smalls.tile([128, 1], F32, tag="den")
                if len(segs) > 1:
                    nc.vector.reduce_sum(den, acc[:, 0:len(segs)], axis=mybir.AxisListType.X)
                else:
                    nc.vector.tensor_copy(out=den, in_=acc[:, 0:1])
                rd = smalls.tile([128, 1], F32, tag="rd")
                nc.vector.reciprocal(rd, den)
                vprime = smalls.tile([128, KS], F16, tag="vprime")
                nc.vector.tensor_scalar_mul(
                    out=vprime, in0=v_all[:, KS * it:KS * (it + 1)], scalar1=rd
                )
                pending = (it, vprime, P)
            emit_pv(*pending)

            # ---- epilogue: transpose outT back to [t, 16] and finish rows ----
            oT = osb.tile([128, 512], F16, tag="oT")
            nc.vector.tensor_copy(out=oT, in_=outT)
            for n in range(NT):
                jc, blk = n // 4, n % 4
                po = ps.tile([128, KS], F16, tag="st", name="po")
                nc.tensor.matmul(
                    po,
                    lhsT=oT[32 * jc:32 * jc + KS, 128 * blk:128 * (blk + 1)],
                    rhs=idh_sb[32 * jc:32 * jc + KS, 32 * jc:32 * jc + KS],
                    is_transpose=True,
                    tile_position=(32 * jc, 0),
                )
                nc.vector.tensor_copy(out=OUTv[:, n, C:OUTW], in_=po)
            nc.sync.dma_start(
                out=y_t[b].rearrange("(n p) f -> p n f", p=128),
                in_=OUTv,
            )

    nc.compile()
    return nc


def _host_params(Wq, bq, Wk, bk, Wv, bv):
    wq = np.zeros((C, 128), dtype=np.float16)
    wq[:, 0:KS] = Wq.T            # q replica 0 (partitions 0-15)
    wq[:, 32:48] = Wv.T           # vT (partitions 32-47)
    wq[:, 64:80] = Wq.T           # q replica 1 (partitions 64-79)
    wk = np.zeros((C, 128), dtype=np.float16)
    wk[:, 0:KS] = Wk.T
    wk[:, 64:80] = Wk.T
    bq128 = np.zeros((128, 1), dtype=np.float32)
    bq128[32:48, 0] = bv          # v bias rides the q-projection copy
    bk128 = np.zeros((128, 1), dtype=np.float32)
    bk128[0:KS, 0] = bk
    bk128[64:80, 0] = bk
    p = np.arange(128)[:, None]
    f = np.arange(128)[None, :]
    mneg = np.where(f < p, np.float16(NEG), np.float16(0.0)).astype(np.float16)
    idh = np.eye(128, dtype=np.float16)
    return wq, wk, bq128, bk128, mneg, idh


def run(inputs, Wq, bq, Wk, bk, Wv, bv, trace=False):
    """Run on 8 cores; returns (out [B,T,OUTW], BassKernelResults)."""
    if "nc" not in _cache:
        _cache["nc"] = _build_program()
    nc = _cache["nc"]
    wq, wk, bq128, bk128, mneg, idh = _host_params(
        np.asarray(Wq), np.asarray(bq), np.asarray(Wk),
        np.asarray(bk), np.asarray(Wv), np.asarray(bv))
    x = np.ascontiguousarray(np.asarray(inputs), dtype=np.float32)
    in_maps = []
    for core in range(NCORES):
        in_maps.append({
            "x": x[NB * core: NB * (core + 1)],
            "wq": wq, "wk": wk, "bq128": bq128, "bk128": bk128,
            "mneg": mneg, "idh": idh,
        })
    res = run_bass_kernel_spmd(nc, in_maps, core_ids=list(range(NCORES)), trace=trace)
    out = np.concatenate([res.results[i]["y"] for i in range(NCORES)], axis=0)
    return out, res


def kernel(inputs, Wq, bq, Wk, bk, Wv, bv):
    out, _ = run(inputs, Wq, bq, Wk, bk, Wv, bv, trace=False)
    return out
